# revision 32
# baseline (speedup 1.0000x reference)
"""Trainium2 Bass kernel for nn_MultiHeadAttention_28819230556860.

SimA (softmax-free) multi-head attention:
    q,k,v = per-head projections of x; q,k L2-normalized over the token axis;
    out = (tril(q k^T) * C**-0.5) v, heads concatenated, projected by Wp + bp.

Because there is no softmax, the causal attention is computed in linear
(cumulative-state) form: for each 128-token block,
    out_blk = q_blk^T S + tril_mask(q_blk^T k_blk) v_blk,   S += k_blk v_blk^T
which avoids materializing the 2048x2048 score matrix.

Sharding: 8 cores = 4 batches x 2 head-groups (6 heads each).  Each core
computes a partial (T, C) output (row-sharded Wp); the host sums core pairs.
"""

import os
import sys

sys.path.insert(0, "/opt/trn_rl_repo")

from contextlib import ExitStack

import ml_dtypes
import numpy as np

import concourse.bass as bass
import concourse.tile as tile
from concourse import bacc, mybir

# ---------------------------------------------------------------------------
# Problem constants (hardcoded from the reference nn.Module).
# ---------------------------------------------------------------------------
B, T, C = 4, 2048, 768
H, D = 12, 64
P = 128
KC = C // P          # 6 contraction tiles over the embedding dim
NP = 3               # head-pairs per core (6 heads, 2 per 128 partitions)
TW = 512             # t-window for 512-wide psum tiles
NTW = T // TW        # 4
NTB = T // P         # 16 token blocks
SCALE = float(C) ** -0.5
EPS = 1e-12

F32 = mybir.dt.float32


def build_nc(dt=F32, qk_bufs=1, skip=()):
    """Build the per-core Bass program (same program on all 8 cores).
    skip: feature names to disable, for hardware bisection."""
    nc = bacc.Bacc("TRN2", target_bir_lowering=False, debug=False,
                   enable_asserts=False)

    xT = nc.dram_tensor("xT", [C, T], dt, kind="ExternalInput").ap()
    wq = nc.dram_tensor("wq", [C, NP * P], dt, kind="ExternalInput").ap()
    wk = nc.dram_tensor("wk", [C, NP * P], dt, kind="ExternalInput").ap()
    wv = nc.dram_tensor("wv", [C, NP * P], dt, kind="ExternalInput").ap()
    wp = nc.dram_tensor("wp", [NP * P, C], dt, kind="ExternalInput").ap()
    mask = nc.dram_tensor("mask", [P, P], F32, kind="ExternalInput").ap()
    out = nc.dram_tensor("out", [T, C], F32, kind="ExternalOutput").ap()

    with tile.TileContext(nc) as tc:
        with ExitStack() as ctx:
            _body(ctx, tc, dt, qk_bufs, skip, xT, wq, wk, wv, wp, mask, out)
    nc.compile()
    return nc


def _body(ctx, tc, dt, qk_bufs, skip, xT, wq, wk, wv, wp, mask, out):
    nc = tc.nc
    AX = mybir.AxisListType
    OP = mybir.AluOpType
    AF = mybir.ActivationFunctionType

    consts = ctx.enter_context(tc.tile_pool(name="consts", bufs=1))

    xt_t = []
    wq_t, wk_t, wv_t = [], [], []
    for kc in range(KC):
        csl = slice(kc * P, (kc + 1) * P)
        t_ = consts.tile([P, T], dt, name=f"xt{kc}", tag=f"xt{kc}")
        nc.sync.dma_start(t_[:, 0:256], xT[csl, 0:256])
        xt_t.append(t_)
        w_ = consts.tile([P, NP * P], dt, name=f"wv{kc}", tag=f"wv{kc}")
        nc.sync.dma_start(w_[:], wv[csl, :])
        wv_t.append(w_)
        w_ = consts.tile([P, NP * P], dt, name=f"wk{kc}", tag=f"wk{kc}")
        nc.sync.dma_start(w_[:], wk[csl, :])
        wk_t.append(w_)
    for kc in range(KC):
        nc.sync.dma_start(xt_t[kc][:, 256:T // 2], xT[kc * P:(kc + 1) * P, 256:T // 2])
    for kc in range(KC):
        nc.sync.dma_start(xt_t[kc][:, T // 2:T], xT[kc * P:(kc + 1) * P, T // 2:T])
    for kc in range(KC):
        csl = slice(kc * P, (kc + 1) * P)
        w_ = consts.tile([P, NP * P], dt, name=f"wq{kc}", tag=f"wq{kc}")
        nc.sync.dma_start(w_[:], wq[csl, :])
        wq_t.append(w_)
    wp_t = []
    for p in range(NP):
        w_ = consts.tile([P, C], dt, name=f"wp{p}", tag=f"wp{p}")
        nc.sync.dma_start(w_[:], wp[p * P:(p + 1) * P, :])
        wp_t.append(w_)
    mask_sb = consts.tile([P, P], F32, name="mask_sb", tag="mask_sb")
    nc.sync.dma_start(mask_sb[:], mask[:])
    scale_sb = consts.tile([P, 1], F32, name="scale_sb", tag="scale_sb")
    nc.vector.memset(scale_sb[:], SCALE)

    # PSUM pools: 8 banks total.  mm: stage-1/proj accumulators; wei: intra
    # score blocks; S: cumulative state; out: attention output accumulator.
    ps_mm = ctx.enter_context(tc.tile_pool(name="ps_mm", bufs=2, space="PSUM"))
    ps_wei = ctx.enter_context(tc.tile_pool(name="ps_wei", bufs=3, space="PSUM"))
    ps_S = ctx.enter_context(tc.tile_pool(name="ps_S", bufs=1, space="PSUM"))
    ps_out = ctx.enter_context(tc.tile_pool(name="ps_out", bufs=2, space="PSUM"))

    vpool = ctx.enter_context(tc.tile_pool(name="vpool", bufs=1))
    qkpool = ctx.enter_context(tc.tile_pool(name="qkpool", bufs=qk_bufs))
    spool = ctx.enter_context(tc.tile_pool(name="spool", bufs=2))
    opool = ctx.enter_context(tc.tile_pool(name="opool", bufs=1))
    prpool = ctx.enter_context(tc.tile_pool(name="prpool", bufs=3))

    # ---- phase V: v and k_nat (token-major) for all pairs --------------
    v_t, kn_t = [], []
    for tb in range(NTB if "phasev" not in skip else 0):
        tsl = slice(tb * P, (tb + 1) * P)
        pv = ps_mm.tile([P, TW], F32, name="pv", tag="mm")[:, :NP * P]
        for kc in range(KC):
            nc.tensor.matmul(pv, lhsT=xt_t[kc][:, tsl], rhs=wv_t[kc][:],
                             start=(kc == 0), stop=(kc == KC - 1))
        v_ = vpool.tile([P, NP * P], dt, name=f"v{tb}", tag=f"v{tb}")
        (nc.vector.tensor_copy if tb % 2 else nc.scalar.copy)(out=v_[:], in_=pv)
        v_t.append(v_)

        pk = ps_mm.tile([P, TW], F32, name="pk", tag="mm")[:, :NP * P]
        for kc in range(KC):
            nc.tensor.matmul(pk, lhsT=xt_t[kc][:, tsl], rhs=wk_t[kc][:],
                             start=(kc == 0), stop=(kc == KC - 1))
        kn_ = vpool.tile([P, NP * P], dt, name=f"kn{tb}", tag=f"kn{tb}")
        (nc.scalar.copy if tb % 2 else nc.vector.tensor_copy)(out=kn_[:], in_=pk)
        kn_t.append(kn_)

    outT_t = []
    for p in range(NP):
        # ---- stage 1: qT, kT (head-dim-major) for this pair ------------
        qT = qkpool.tile([P, T], dt, name=f"qT{p}", tag="qT")
        kT = qkpool.tile([P, T], dt, name=f"kT{p}", tag="kT")
        psl = slice(p * P, (p + 1) * P)
        sqp = spool.tile([P, NTW], F32, name="sqp", tag="sqp")
        skp = spool.tile([P, NTW], F32, name="skp", tag="skp")
        for tw in range(NTW if "qk" not in skip else 0):
            wsl = slice(tw * TW, (tw + 1) * TW)
            pq = ps_mm.tile([P, TW], F32, name="pq", tag="mm")
            for kc in range(KC):
                nc.tensor.matmul(pq, lhsT=wq_t[kc][:, psl],
                                 rhs=xt_t[kc][:, wsl],
                                 start=(kc == 0), stop=(kc == KC - 1))
            nc.vector.tensor_copy(out=qT[:, wsl], in_=pq)
            scr = spool.tile([P, TW], F32, name="scr", tag="scr")
            nc.scalar.activation(out=scr[:], in_=pq, func=AF.Square,
                                 accum_out=sqp[:, tw:tw + 1])
            pk2 = ps_mm.tile([P, TW], F32, name="pk2", tag="mm")
            for kc in range(KC):
                nc.tensor.matmul(pk2, lhsT=wk_t[kc][:, psl],
                                 rhs=xt_t[kc][:, wsl],
                                 start=(kc == 0), stop=(kc == KC - 1))
            nc.vector.tensor_copy(out=kT[:, wsl], in_=pk2)
            scr2 = spool.tile([P, TW], F32, name="scr2", tag="scr")
            nc.scalar.activation(out=scr2[:], in_=pk2, func=AF.Square,
                                 accum_out=skp[:, tw:tw + 1])

        # ---- normalization factors -------------------------------------
        # factor[d] = SCALE / (max(||q_d||, eps) * max(||k_d||, eps)),
        # applied once to qT (equivalent to normalizing both q and k and
        # scaling the scores).
        if "norm" in skip:
            outT = opool.tile([P, T], dt, name=f"outT{p}", tag=f"outT{p}")
            outT_t.append(outT)
            continue
        fac = spool.tile([P, 1], F32, name="fac", tag="fac")
        if "facops" not in skip:
            nq = spool.tile([P, 1], F32, name="nq", tag="nq")
            nk = spool.tile([P, 1], F32, name="nk", tag="nk")
            scr4 = spool.tile([P, NTW], F32, name="scr4", tag="scr4")
            nc.scalar.activation(out=scr4[:], in_=sqp[:], func=AF.Copy,
                                 accum_out=nq[:])
            scr5 = spool.tile([P, NTW], F32, name="scr5", tag="scr4")
            nc.scalar.activation(out=scr5[:], in_=skp[:], func=AF.Copy,
                                 accum_out=nk[:])
            nc.scalar.sqrt(nq[:], nq[:])
            nc.scalar.sqrt(nk[:], nk[:])
            nc.vector.tensor_scalar_max(nq[:], nq[:], EPS)
            nc.vector.tensor_scalar_max(nk[:], nk[:], EPS)
            nc.vector.tensor_mul(fac[:], nq[:], nk[:])
            nc.vector.reciprocal(fac[:], fac[:])
            nc.vector.tensor_scalar_mul(fac[:], fac[:], SCALE)
        if "qscale" not in skip:
            for tw in range(NTW):
                wsl = slice(tw * TW, (tw + 1) * TW)
                if tw % 2:
                    nc.scalar.mul(out=qT[:, wsl], in_=qT[:, wsl], mul=fac[:])
                else:
                    nc.vector.tensor_scalar_mul(qT[:, wsl], qT[:, wsl], fac[:])

        # ---- attention (linear/cumulative form) ------------------------
        pS = ps_S.tile([P, D], F32, name="pS", tag="S")
        outT = opool.tile([P, T], dt, name=f"outT{p}", tag=f"outT{p}")
        if "attn" in skip:
            outT_t.append(outT)
            continue
        for tw in range(NTW):
            po = ps_out.tile([P, TW], F32, name="po", tag="out")
            for tb4 in range(4):
                tb = tw * 4 + tb4
                tsl = slice(tb * P, (tb + 1) * P)
                osl = slice(tb4 * P, (tb4 + 1) * P)
                vA = v_t[tb][:, p * P:p * P + D]
                vB = v_t[tb][:, p * P + D:(p + 1) * P]
                knA = kn_t[tb][:, p * P:p * P + D]
                knB = kn_t[tb][:, p * P + D:(p + 1) * P]

                if tb > 0 and "snap" not in skip:
                    S_sb = spool.tile([P, D], dt, name="S_sb", tag="S_sb",
                                      bufs=4)
                    nc.vector.tensor_copy(out=S_sb[:], in_=pS[:])

                # intra-block masked scores (both heads, row-packed)
                pwA = ps_wei.tile([P, P], F32, name="pwA", tag="wei")
                nc.tensor.matmul(pwA, lhsT=kT[0:D, tsl], rhs=qT[0:D, tsl],
                                 start=True, stop=True)
                pwB = ps_wei.tile([P, P], F32, name="pwB", tag="wei")
                nc.tensor.matmul(pwB, lhsT=kT[D:P, tsl], rhs=qT[D:P, tsl],
                                 start=True, stop=True)
                wA = spool.tile([P, P], dt, name="wA", tag="wsb", bufs=8)
                nc.vector.tensor_tensor(out=wA[:], in0=pwA, in1=mask_sb[:],
                                        op=OP.mult)
                wB = spool.tile([P, P], dt, name="wB", tag="wsb", bufs=8)
                nc.vector.tensor_tensor(out=wB[:], in0=pwB, in1=mask_sb[:],
                                        op=OP.mult)

                # out += v_blk^T wei  (intra), then += S^T q_blk (inter)
                closeA = (tb == 0) or "snap" in skip or "interA" in skip
                nc.tensor.matmul(po[0:D, osl], lhsT=vA, rhs=wA[:],
                                 start=True, stop=closeA,
                                 skip_group_check=True)
                closeB = (tb == 0) or "snap" in skip or "interB" in skip
                nc.tensor.matmul(po[D:P, osl], lhsT=vB, rhs=wB[:],
                                 start=True, stop=closeB,
                                 skip_group_check=True)
                if tb > 0 and "snap" not in skip and "interA" not in skip:
                    nc.tensor.matmul(po[0:D, osl], lhsT=S_sb[0:D, :],
                                     rhs=qT[0:D, tsl], start=False, stop=True,
                                     skip_group_check=True)
                if tb > 0 and "snap" not in skip and "interB" not in skip:
                    nc.tensor.matmul(po[D:P, osl], lhsT=S_sb[D:P, :],
                                     rhs=qT[D:P, tsl], start=False, stop=True,
                                     skip_group_check=True)

                # state update S += k_blk^T v_blk (skip last, never read)
                if tb < NTB - 1 and "state" not in skip:
                    nc.tensor.matmul(pS[0:D, :], lhsT=knA, rhs=vA,
                                     start=(tb == 0), stop=(tb == NTB - 2),
                                     skip_group_check=True)
                    nc.tensor.matmul(pS[D:P, :], lhsT=knB, rhs=vB,
                                     start=(tb == 0), stop=(tb == NTB - 2),
                                     skip_group_check=True)
            wsl = slice(tw * TW, (tw + 1) * TW)
            (nc.scalar.copy if tw % 2 else nc.vector.tensor_copy)(
                out=outT[:, wsl], in_=po)
        outT_t.append(outT)

    # ---- output projection + bias --------------------------------------
    for tb in range(NTB if "proj" not in skip else 0):
        tsl = slice(tb * P, (tb + 1) * P)
        pr = prpool.tile([P, C], F32, name="pr", tag="pr")
        for (n0, nsz) in ((0, TW), (TW, C - TW)):
            pp = ps_mm.tile([P, TW], F32, name="pp", tag="mm")[:, :nsz]
            for p in range(NP):
                nc.tensor.matmul(pp, lhsT=outT_t[p][:, tsl],
                                 rhs=wp_t[p][:, n0:n0 + nsz],
                                 start=(p == 0), stop=(p == NP - 1))
            (nc.scalar.copy if (tb + (n0 > 0)) % 2 else nc.vector.tensor_copy)(
                out=pr[:, n0:n0 + nsz], in_=pp)
        nc.sync.dma_start(out[tsl, :], pr[:])


# ---------------------------------------------------------------------------
# Host side: shard, run on 8 cores, unshard.
# ---------------------------------------------------------------------------

_DT_NAME = os.environ.get("KERNEL_DT", "bf16")
DT = {"f32": mybir.dt.float32, "bf16": mybir.dt.bfloat16,
      "f32r": mybir.dt.float32r}[_DT_NAME]
_NP_DT = {"f32": np.float32, "bf16": ml_dtypes.bfloat16,
          "f32r": np.float32}[_DT_NAME]

_CACHED = {}


def _get_nc():
    key = (DT, )
    if key not in _CACHED:
        _CACHED[key] = build_nc(DT, qk_bufs=1 if DT == F32 else 2)
    return _CACHED[key]


def make_in_maps(x, Wq, Wk, Wv, Wp, bp):
    x = np.asarray(x, np.float32)
    Wq = np.asarray(Wq, np.float32)
    Wk = np.asarray(Wk, np.float32)
    Wv = np.asarray(Wv, np.float32)
    Wp = np.asarray(Wp, np.float32)
    bp = np.asarray(bp, np.float32)

    cast = lambda a: np.ascontiguousarray(a).astype(_NP_DT)
    # mask[s, t] = 1 where t >= s (keep, causal incl. diagonal)
    mask = np.triu(np.ones((P, P), np.float32))
    HG = H // 2  # heads per group

    in_maps = []
    for core in range(8):
        b, g = divmod(core, 2)
        hsl = slice(g * HG, (g + 1) * HG)
        wq_s = Wq[hsl].transpose(1, 0, 2).reshape(C, HG * D)
        wk_s = Wk[hsl].transpose(1, 0, 2).reshape(C, HG * D)
        wv_s = Wv[hsl].transpose(1, 0, 2).reshape(C, HG * D)
        wp_s = Wp[g * HG * D:(g + 1) * HG * D, :]
        in_maps.append({
            "xT": cast(x[b].T),
            "wq": cast(wq_s),
            "wk": cast(wk_s),
            "wv": cast(wv_s),
            "wp": cast(wp_s),
            "mask": mask,
        })
    return in_maps


def kernel(x, Wq, Wk, Wv, Wp, bp):
    from concourse.bass_utils import run_bass_kernel_spmd

    in_maps = make_in_maps(x, Wq, Wk, Wv, Wp, bp)
    nc = _get_nc()
    res = run_bass_kernel_spmd(nc, in_maps, core_ids=list(range(8)))
    parts = [r["out"] for r in res.results]
    bp32 = np.asarray(bp, np.float32)
    return np.stack([parts[2 * b] + parts[2 * b + 1] + bp32 for b in range(B)])


# revision 33
# speedup vs baseline: 1.0457x; 1.0457x over previous
"""Trainium2 Bass kernel for nn_MultiHeadAttention_28819230556860.

SimA (softmax-free) multi-head attention:
    q,k,v = per-head projections of x; q,k L2-normalized over the token axis;
    out = (tril(q k^T) * C**-0.5) v, heads concatenated, projected by Wp + bp.

Because there is no softmax, the causal attention is computed in linear
(cumulative-state) form: for each 128-token block,
    out_blk = q_blk^T S + tril_mask(q_blk^T k_blk) v_blk,   S += k_blk v_blk^T
which avoids materializing the 2048x2048 score matrix.

Sharding: 8 cores = 4 batches x 2 head-groups (6 heads each).  Each core
computes a partial (T, C) output (row-sharded Wp); the host sums core pairs.
"""

import os
import sys

sys.path.insert(0, "/opt/trn_rl_repo")

from contextlib import ExitStack

import ml_dtypes
import numpy as np

import concourse.bass as bass
import concourse.tile as tile
from concourse import bacc, mybir

# ---------------------------------------------------------------------------
# Problem constants (hardcoded from the reference nn.Module).
# ---------------------------------------------------------------------------
B, T, C = 4, 2048, 768
H, D = 12, 64
P = 128
KC = C // P          # 6 contraction tiles over the embedding dim
NP = 3               # head-pairs per core (6 heads, 2 per 128 partitions)
TW = 512             # t-window for 512-wide psum tiles
NTW = T // TW        # 4
NTB = T // P         # 16 token blocks
SCALE = float(C) ** -0.5
EPS = 1e-12

F32 = mybir.dt.float32


def build_nc(dt=F32, qk_bufs=1, skip=()):
    """Build the per-core Bass program (same program on all 8 cores).
    skip: feature names to disable, for hardware bisection."""
    nc = bacc.Bacc("TRN2", target_bir_lowering=False, debug=False,
                   enable_asserts=False)

    xT = nc.dram_tensor("xT", [C, T], dt, kind="ExternalInput").ap()
    wq = nc.dram_tensor("wq", [C, NP * P], dt, kind="ExternalInput").ap()
    wk = nc.dram_tensor("wk", [C, NP * P], dt, kind="ExternalInput").ap()
    wv = nc.dram_tensor("wv", [C, NP * P], dt, kind="ExternalInput").ap()
    wp = nc.dram_tensor("wp", [NP * P, C], dt, kind="ExternalInput").ap()
    mask = nc.dram_tensor("mask", [P, P], F32, kind="ExternalInput").ap()
    out = nc.dram_tensor("out", [T, C], F32, kind="ExternalOutput").ap()

    with tile.TileContext(nc) as tc:
        with ExitStack() as ctx:
            _body(ctx, tc, dt, qk_bufs, skip, xT, wq, wk, wv, wp, mask, out)
    nc.compile()
    return nc


def _body(ctx, tc, dt, qk_bufs, skip, xT, wq, wk, wv, wp, mask, out):
    nc = tc.nc
    AX = mybir.AxisListType
    OP = mybir.AluOpType
    AF = mybir.ActivationFunctionType

    consts = ctx.enter_context(tc.tile_pool(name="consts", bufs=1))

    xt_t = []
    wq_t, wk_t, wv_t = [], [], []
    for kc in range(KC):
        csl = slice(kc * P, (kc + 1) * P)
        t_ = consts.tile([P, T], dt, name=f"xt{kc}", tag=f"xt{kc}")
        nc.sync.dma_start(t_[:, 0:256], xT[csl, 0:256])
        xt_t.append(t_)
        w_ = consts.tile([P, NP * P], dt, name=f"wv{kc}", tag=f"wv{kc}")
        nc.sync.dma_start(w_[:], wv[csl, :])
        wv_t.append(w_)
        w_ = consts.tile([P, NP * P], dt, name=f"wk{kc}", tag=f"wk{kc}")
        nc.sync.dma_start(w_[:], wk[csl, :])
        wk_t.append(w_)
    for kc in range(KC):
        nc.sync.dma_start(xt_t[kc][:, 256:T // 2], xT[kc * P:(kc + 1) * P, 256:T // 2])
    for kc in range(KC):
        nc.sync.dma_start(xt_t[kc][:, T // 2:T], xT[kc * P:(kc + 1) * P, T // 2:T])
    for kc in range(KC):
        csl = slice(kc * P, (kc + 1) * P)
        w_ = consts.tile([P, NP * P], dt, name=f"wq{kc}", tag=f"wq{kc}")
        nc.sync.dma_start(w_[:], wq[csl, :])
        wq_t.append(w_)
    wp_t = []
    for p in range(NP):
        w_ = consts.tile([P, C], dt, name=f"wp{p}", tag=f"wp{p}")
        nc.sync.dma_start(w_[:], wp[p * P:(p + 1) * P, :])
        wp_t.append(w_)
    mask_sb = consts.tile([P, P], F32, name="mask_sb", tag="mask_sb")
    nc.sync.dma_start(mask_sb[:], mask[:])
    scale_sb = consts.tile([P, 1], F32, name="scale_sb", tag="scale_sb")
    nc.vector.memset(scale_sb[:], SCALE)

    # PSUM pools: 8 banks total.  mm: stage-1/proj accumulators; wei: intra
    # score blocks; S: cumulative state; out: attention output accumulator.
    ps_mm = ctx.enter_context(tc.tile_pool(name="ps_mm", bufs=2, space="PSUM"))

    vpool = ctx.enter_context(tc.tile_pool(name="vpool", bufs=1))
    qkpool = ctx.enter_context(tc.tile_pool(name="qkpool", bufs=qk_bufs))
    spool = ctx.enter_context(tc.tile_pool(name="spool", bufs=2))
    opool = ctx.enter_context(tc.tile_pool(name="opool", bufs=1))
    prpool = ctx.enter_context(tc.tile_pool(name="prpool", bufs=4))

    # ---- phase V: v and k_nat (token-major) for all pairs --------------
    v_t, kn_t = [], []
    for tb in range(NTB if "phasev" not in skip else 0):
        tsl = slice(tb * P, (tb + 1) * P)
        pv = ps_mm.tile([P, TW], F32, name="pv", tag="mm")[:, :NP * P]
        for kc in range(KC):
            nc.tensor.matmul(pv, lhsT=xt_t[kc][:, tsl], rhs=wv_t[kc][:],
                             start=(kc == 0), stop=(kc == KC - 1))
        v_ = vpool.tile([P, NP * P], dt, name=f"v{tb}", tag=f"v{tb}")
        (nc.vector.tensor_copy if tb % 2 else nc.scalar.copy)(out=v_[:], in_=pv)
        v_t.append(v_)

        pk = ps_mm.tile([P, TW], F32, name="pk", tag="mm")[:, :NP * P]
        for kc in range(KC):
            nc.tensor.matmul(pk, lhsT=xt_t[kc][:, tsl], rhs=wk_t[kc][:],
                             start=(kc == 0), stop=(kc == KC - 1))
        kn_ = vpool.tile([P, NP * P], dt, name=f"kn{tb}", tag=f"kn{tb}")
        (nc.scalar.copy if tb % 2 else nc.vector.tensor_copy)(out=kn_[:], in_=pk)
        kn_t.append(kn_)

    outT_t = []
    attn_ctx = ExitStack()
    ps_wei = attn_ctx.enter_context(
        tc.tile_pool(name="ps_wei", bufs=3, space="PSUM"))
    ps_S = attn_ctx.enter_context(
        tc.tile_pool(name="ps_S", bufs=1, space="PSUM"))
    ps_out = attn_ctx.enter_context(
        tc.tile_pool(name="ps_out", bufs=2, space="PSUM"))
    for p in range(NP):
        # ---- stage 1: qT, kT (head-dim-major) for this pair ------------
        qT = qkpool.tile([P, T], dt, name=f"qT{p}", tag="qT")
        kT = qkpool.tile([P, T], dt, name=f"kT{p}", tag="kT")
        psl = slice(p * P, (p + 1) * P)
        sqp = spool.tile([P, NTW], F32, name="sqp", tag="sqp")
        skp = spool.tile([P, NTW], F32, name="skp", tag="skp")
        for tw in range(NTW if "qk" not in skip else 0):
            wsl = slice(tw * TW, (tw + 1) * TW)
            pq = ps_mm.tile([P, TW], F32, name="pq", tag="mm")
            for kc in range(KC):
                nc.tensor.matmul(pq, lhsT=wq_t[kc][:, psl],
                                 rhs=xt_t[kc][:, wsl],
                                 start=(kc == 0), stop=(kc == KC - 1))
            nc.vector.tensor_copy(out=qT[:, wsl], in_=pq)
            scr = spool.tile([P, TW], F32, name="scr", tag="scr")
            nc.scalar.activation(out=scr[:], in_=pq, func=AF.Square,
                                 accum_out=sqp[:, tw:tw + 1])
            pk2 = ps_mm.tile([P, TW], F32, name="pk2", tag="mm")
            for kc in range(KC):
                nc.tensor.matmul(pk2, lhsT=wk_t[kc][:, psl],
                                 rhs=xt_t[kc][:, wsl],
                                 start=(kc == 0), stop=(kc == KC - 1))
            nc.vector.tensor_copy(out=kT[:, wsl], in_=pk2)
            scr2 = spool.tile([P, TW], F32, name="scr2", tag="scr")
            nc.scalar.activation(out=scr2[:], in_=pk2, func=AF.Square,
                                 accum_out=skp[:, tw:tw + 1])

        # ---- normalization factors -------------------------------------
        # factor[d] = SCALE / (max(||q_d||, eps) * max(||k_d||, eps)),
        # applied once to qT (equivalent to normalizing both q and k and
        # scaling the scores).
        if "norm" in skip:
            outT = opool.tile([P, T], dt, name=f"outT{p}", tag=f"outT{p}")
            outT_t.append(outT)
            continue
        fac = spool.tile([P, 1], F32, name="fac", tag="fac")
        if "facops" not in skip:
            nq = spool.tile([P, 1], F32, name="nq", tag="nq")
            nk = spool.tile([P, 1], F32, name="nk", tag="nk")
            scr4 = spool.tile([P, NTW], F32, name="scr4", tag="scr4")
            nc.scalar.activation(out=scr4[:], in_=sqp[:], func=AF.Copy,
                                 accum_out=nq[:])
            scr5 = spool.tile([P, NTW], F32, name="scr5", tag="scr4")
            nc.scalar.activation(out=scr5[:], in_=skp[:], func=AF.Copy,
                                 accum_out=nk[:])
            nc.scalar.sqrt(nq[:], nq[:])
            nc.scalar.sqrt(nk[:], nk[:])
            nc.vector.tensor_scalar_max(nq[:], nq[:], EPS)
            nc.vector.tensor_scalar_max(nk[:], nk[:], EPS)
            nc.vector.tensor_mul(fac[:], nq[:], nk[:])
            nc.vector.reciprocal(fac[:], fac[:])
            nc.vector.tensor_scalar_mul(fac[:], fac[:], SCALE)
        if "qscale" not in skip:
            for tw in range(NTW):
                wsl = slice(tw * TW, (tw + 1) * TW)
                if tw % 2:
                    nc.scalar.mul(out=qT[:, wsl], in_=qT[:, wsl], mul=fac[:])
                else:
                    nc.vector.tensor_scalar_mul(qT[:, wsl], qT[:, wsl], fac[:])

        # ---- attention (linear/cumulative form) ------------------------
        pS = ps_S.tile([P, D], F32, name="pS", tag="S")
        outT = opool.tile([P, T], dt, name=f"outT{p}", tag=f"outT{p}")
        if "attn" in skip:
            outT_t.append(outT)
            continue
        for tw in range(NTW):
            po = ps_out.tile([P, TW], F32, name="po", tag="out")
            for tb4 in range(4):
                tb = tw * 4 + tb4
                tsl = slice(tb * P, (tb + 1) * P)
                osl = slice(tb4 * P, (tb4 + 1) * P)
                vA = v_t[tb][:, p * P:p * P + D]
                vB = v_t[tb][:, p * P + D:(p + 1) * P]
                knA = kn_t[tb][:, p * P:p * P + D]
                knB = kn_t[tb][:, p * P + D:(p + 1) * P]

                if tb > 0 and "snap" not in skip:
                    S_sb = spool.tile([P, D], dt, name="S_sb", tag="S_sb",
                                      bufs=4)
                    nc.vector.tensor_copy(out=S_sb[:], in_=pS[:])

                # intra-block masked scores (both heads, row-packed)
                pwA = ps_wei.tile([P, P], F32, name="pwA", tag="wei")
                nc.tensor.matmul(pwA, lhsT=kT[0:D, tsl], rhs=qT[0:D, tsl],
                                 start=True, stop=True)
                pwB = ps_wei.tile([P, P], F32, name="pwB", tag="wei")
                nc.tensor.matmul(pwB, lhsT=kT[D:P, tsl], rhs=qT[D:P, tsl],
                                 start=True, stop=True)
                wA = spool.tile([P, P], dt, name="wA", tag="wsb", bufs=8)
                nc.vector.tensor_tensor(out=wA[:], in0=pwA, in1=mask_sb[:],
                                        op=OP.mult)
                wB = spool.tile([P, P], dt, name="wB", tag="wsb", bufs=8)
                nc.vector.tensor_tensor(out=wB[:], in0=pwB, in1=mask_sb[:],
                                        op=OP.mult)

                # out += v_blk^T wei  (intra), then += S^T q_blk (inter)
                closeA = (tb == 0) or "snap" in skip or "interA" in skip
                nc.tensor.matmul(po[0:D, osl], lhsT=vA, rhs=wA[:],
                                 start=True, stop=closeA,
                                 skip_group_check=True)
                closeB = (tb == 0) or "snap" in skip or "interB" in skip
                nc.tensor.matmul(po[D:P, osl], lhsT=vB, rhs=wB[:],
                                 start=True, stop=closeB,
                                 skip_group_check=True)
                if tb > 0 and "snap" not in skip and "interA" not in skip:
                    nc.tensor.matmul(po[0:D, osl], lhsT=S_sb[0:D, :],
                                     rhs=qT[0:D, tsl], start=False, stop=True,
                                     skip_group_check=True)
                if tb > 0 and "snap" not in skip and "interB" not in skip:
                    nc.tensor.matmul(po[D:P, osl], lhsT=S_sb[D:P, :],
                                     rhs=qT[D:P, tsl], start=False, stop=True,
                                     skip_group_check=True)

                # state update S += k_blk^T v_blk (skip last, never read)
                if tb < NTB - 1 and "state" not in skip:
                    nc.tensor.matmul(pS[0:D, :], lhsT=knA, rhs=vA,
                                     start=(tb == 0), stop=(tb == NTB - 2),
                                     skip_group_check=True)
                    nc.tensor.matmul(pS[D:P, :], lhsT=knB, rhs=vB,
                                     start=(tb == 0), stop=(tb == NTB - 2),
                                     skip_group_check=True)
            wsl = slice(tw * TW, (tw + 1) * TW)
            (nc.scalar.copy if tw % 2 else nc.vector.tensor_copy)(
                out=outT[:, wsl], in_=po)
        outT_t.append(outT)

    # ---- output projection -------------------------------------------
    attn_ctx.close()
    ps_pr = ctx.enter_context(tc.tile_pool(name="ps_pr", bufs=4, space="PSUM"))
    for tb in range(NTB if "proj" not in skip else 0):
        tsl = slice(tb * P, (tb + 1) * P)
        pr = prpool.tile([P, C], F32, name="pr", tag="pr")
        for (n0, nsz) in ((0, TW), (TW, C - TW)):
            pp = ps_pr.tile([P, TW], F32, name="pp", tag="pr")[:, :nsz]
            for p in range(NP):
                nc.tensor.matmul(pp, lhsT=outT_t[p][:, tsl],
                                 rhs=wp_t[p][:, n0:n0 + nsz],
                                 start=(p == 0), stop=(p == NP - 1))
            (nc.scalar.copy if (tb + (n0 > 0)) % 2 else nc.vector.tensor_copy)(
                out=pr[:, n0:n0 + nsz], in_=pp)
        nc.sync.dma_start(out[tsl, :], pr[:])


# ---------------------------------------------------------------------------
# Host side: shard, run on 8 cores, unshard.
# ---------------------------------------------------------------------------

_DT_NAME = os.environ.get("KERNEL_DT", "bf16")
DT = {"f32": mybir.dt.float32, "bf16": mybir.dt.bfloat16,
      "f32r": mybir.dt.float32r}[_DT_NAME]
_NP_DT = {"f32": np.float32, "bf16": ml_dtypes.bfloat16,
          "f32r": np.float32}[_DT_NAME]

_CACHED = {}


def _get_nc():
    key = (DT, )
    if key not in _CACHED:
        _CACHED[key] = build_nc(DT, qk_bufs=1 if DT == F32 else 2)
    return _CACHED[key]


def make_in_maps(x, Wq, Wk, Wv, Wp, bp):
    x = np.asarray(x, np.float32)
    Wq = np.asarray(Wq, np.float32)
    Wk = np.asarray(Wk, np.float32)
    Wv = np.asarray(Wv, np.float32)
    Wp = np.asarray(Wp, np.float32)
    bp = np.asarray(bp, np.float32)

    cast = lambda a: np.ascontiguousarray(a).astype(_NP_DT)
    # mask[s, t] = 1 where t >= s (keep, causal incl. diagonal)
    mask = np.triu(np.ones((P, P), np.float32))
    HG = H // 2  # heads per group

    in_maps = []
    for core in range(8):
        b, g = divmod(core, 2)
        hsl = slice(g * HG, (g + 1) * HG)
        wq_s = Wq[hsl].transpose(1, 0, 2).reshape(C, HG * D)
        wk_s = Wk[hsl].transpose(1, 0, 2).reshape(C, HG * D)
        wv_s = Wv[hsl].transpose(1, 0, 2).reshape(C, HG * D)
        wp_s = Wp[g * HG * D:(g + 1) * HG * D, :]
        in_maps.append({
            "xT": cast(x[b].T),
            "wq": cast(wq_s),
            "wk": cast(wk_s),
            "wv": cast(wv_s),
            "wp": cast(wp_s),
            "mask": mask,
        })
    return in_maps


def kernel(x, Wq, Wk, Wv, Wp, bp):
    from concourse.bass_utils import run_bass_kernel_spmd

    in_maps = make_in_maps(x, Wq, Wk, Wv, Wp, bp)
    nc = _get_nc()
    res = run_bass_kernel_spmd(nc, in_maps, core_ids=list(range(8)))
    parts = [r["out"] for r in res.results]
    bp32 = np.asarray(bp, np.float32)
    return np.stack([parts[2 * b] + parts[2 * b + 1] + bp32 for b in range(B)])
